# revision 24
# baseline (speedup 1.0000x reference)
"""Trainium2 Bass kernel for nn_Attention_CA (cross-attention, B=4 N=1024 M=2048 C=768 H=12).

Sharding: 8 cores = batch(4) x head-group(2). Each core handles one batch
element and 6 heads; the output projection is row-split per head-group so
each core produces a partial [N, C] output; the host sums the two partials
per batch and adds the bias.

Per-core dataflow (everything transposed so the softmax denominator comes
out of the PV matmul via a ones-column in v and no on-chip transposes are
needed):
  qpT[dd,n]  = q @ WqT (pre-scaled), kT[dd,m], v[m,dd]  (PE, bf16 in / f32 acc)
  sT[m,n]    = kT_h^T qpT_h  [128m x 512n] tiles        (PE bf16, f32 PSUM,
               2 heads packed in PE row-groups 0-63/64-127)
  eT         = exp(sT)                                  (ACT, PSUM->SBUF, bf16)
  eTm        = eT * maskT                               (DVE, bf16 2x mode)
  xT_aug     = [v|1]^T eTm  (row 64 = softmax denom)    (PE bf16, f32 acc)
  xTn        = xT * bcast(1/denom)                      (DVE approx-recip +
                                                         SWDGE partition bcast)
  out_part   = xTn^T @ WprojT_hg                        (PE, fp32r)
"""

import numpy as np

DIM = 768
NUM_HEADS = 12
HEAD_DIM = DIM // NUM_HEADS
SCALE = HEAD_DIM ** -0.5
B, N, M, C = 4, 1024, 2048, DIM
HPC = 6          # heads per core
DH = HPC * HEAD_DIM  # 384
NCH = 2          # n halves
NHW = N // NCH   # 512
MC = M // 128    # 16 m chunks

_CACHE = {}


def _apply_tile_patch():
    """Walrus in this container rejects >1 sem wait per instruction; the
    TileContext tail drain attaches the whole global clock. Spread the waits
    across a chain of sync nops instead."""
    import concourse.tile as _tile
    from concourse.vector_clock import ScopedClock, VectorClock

    if getattr(_tile.TileContext, "_drain_patched", False):
        return

    def _split_drain_and_barrier(self, tick_clock, wait_clock):
        nc = self.nc
        vc = tick_clock.global_clock
        ticks = list(vc)
        n = len(ticks)
        for i, t in enumerate(ticks):
            if t > 0:
                sub = [0] * n
                sub[i] = t
                nop = nc.sync.nop(nofuse=True, hint=f"tail_wait_{i}")
                wait_clock.add_sem_waits(nop.ins, ScopedClock({None: VectorClock(sub)}))
        nc.sync.drain()
        nc.all_engine_barrier()
        assert self.sems is not None
        popped = nc._tile_sem_poison_stack.pop()
        assert popped is self._sem_poison
        nc.clear_and_free_semaphores(list(self.sems.allocated().values()))
        nc.all_engine_barrier()

    _tile.TileContext._drain_and_barrier = _split_drain_and_barrier
    _tile.TileContext._drain_patched = True


def _split_excess_waits(nc):
    """This container's walrus accepts at most 1 sync wait per instruction
    (and none on Matmult); move excess waits onto same-engine NoOps."""
    import concourse.mybir as mybir
    for fn in nc.m.functions:
        for bb in fn.blocks:
            insts = bb.instructions
            out = []
            changed = False
            for ins in insts:
                si = ins.sync_info
                maxw = 0 if isinstance(ins, mybir.InstMatmult) else 1
                if si is not None and len(si.on_wait) > maxw:
                    waits = list(si.on_wait)
                    if maxw == 0:
                        extra, keep = waits, []
                    else:
                        extra, keep = waits[:-maxw], waits[-maxw:]
                    for i, w in enumerate(extra):
                        nop = mybir.InstNoOp(name=f"{ins.name}-w{i}", ins=[], outs=[])
                        nop.engine = ins.engine
                        nop.sync_info = mybir.SyncInfo(on_wait=[w], on_update=[])
                        out.append(nop)
                    ins.sync_info = mybir.SyncInfo(
                        on_wait=keep, on_update=list(si.on_update))
                    changed = True
                out.append(ins)
            if changed:
                bb.instructions[:] = out
    return nc


def _build_program(do_split=True):
    import concourse.bass as bass
    import concourse.mybir as mybir
    import concourse.tile as tile

    _apply_tile_patch()

    f32 = mybir.dt.float32
    f32r = mybir.dt.float32r
    bf16 = mybir.dt.bfloat16
    Exp = mybir.ActivationFunctionType.Exp
    Log = mybir.ActivationFunctionType.Ln
    mult = mybir.AluOpType.mult

    nc = bass.Bass()
    qT_d = nc.declare_dram_parameter("qT", [C, N], bf16, isOutput=False)
    kvT_d = nc.declare_dram_parameter("kvT", [C, M], bf16, isOutput=False)
    wq_d = nc.declare_dram_parameter("wqT", [C, DH], bf16, isOutput=False)
    wk_d = nc.declare_dram_parameter("wkT", [C, DH], bf16, isOutput=False)
    wv_d = nc.declare_dram_parameter("wvT", [C, DH], bf16, isOutput=False)
    wp_d = nc.declare_dram_parameter("wpT", [DH, C], bf16, isOutput=False)
    mask_d = nc.declare_dram_parameter("maskT", [M, N], bf16, isOutput=False)
    out_d = nc.declare_dram_parameter("outp", [N, C], f32, isOutput=True)

    with tile.TileContext(nc) as tc:
        # ---- persistent SBUF ----
        maskp = tc.alloc_tile_pool(name="maskp", bufs=1)
        persist = tc.alloc_tile_pool(name="persist", bufs=1)
        wp_pool = tc.alloc_tile_pool(name="wp_pool", bufs=1)
        tmpw = tc.alloc_tile_pool(name="tmpw", bufs=1)
        inp = tc.alloc_tile_pool(name="inp", bufs=1)

        # preload the Exp/Ln ACT table set off the critical path
        warmt = persist.tile([1, 8], f32)
        nc.vector.memset(warmt[:], 0.0)
        nc.scalar.activation(warmt[:], warmt[:], Exp)

        wq_sb = tmpw.tile([128, 6, DH], bf16)
        wk_sb = tmpw.tile([128, 6, DH], bf16)
        wv_sb = tmpw.tile([128, 6, DH], bf16)
        wp_sb = wp_pool.tile([128, 3, C], bf16)
        qT_sb = inp.tile([128, 6, N], bf16)
        kvT_sb = inp.tile([128, 6, M], bf16)

        nc.sync.dma_start(out=wq_sb[:], in_=wq_d.rearrange("(c p) d -> p c d", p=128))
        nc.sync.dma_start(out=qT_sb[:], in_=qT_d.rearrange("(c p) n -> p c n", p=128))
        maskts = []
        for ncx in range(2):
            mt = maskp.tile([128, MC, 2, NHW], bf16, name=f"maskt{ncx}")
            for j in range(2):
                nc.sync.dma_start(
                    out=mt[:, :, j, :],
                    in_=mask_d.rearrange("(mc p) n -> p mc n", p=128)[
                        :, :, ncx * NHW:(ncx + 1) * NHW])
            maskts.append(mt)
        nc.sync.dma_start(out=wk_sb[:], in_=wk_d.rearrange("(c p) d -> p c d", p=128))
        nc.sync.dma_start(out=wv_sb[:], in_=wv_d.rearrange("(c p) d -> p c d", p=128))
        nc.sync.dma_start(out=kvT_sb[:], in_=kvT_d.rearrange("(c p) m -> p c m", p=128))
        nc.sync.dma_start(out=wp_sb[:], in_=wp_d.rearrange("(t p) c -> p t c", p=128))

        qpT = [persist.tile([128, N], bf16, name=f"qpT{t}") for t in range(3)]
        kT = [persist.tile([128, M], bf16, name=f"kT{t}") for t in range(3)]
        vug = [persist.tile([128, HPC, 65], bf16, name=f"vug{m}") for m in range(MC)]
        xTn = [persist.tile([128, N], bf16, name=f"xTn{t}") for t in range(3)]

        # ---- phase A: projections (bf16 in, f32 accumulate) ----
        psA = tc.alloc_tile_pool(name="psA", bufs=2, space="PSUM")
        for t in range(3):
            ps = psA.tile([128, N], f32, tag="proj", padded_shape=[128, M])
            for nh in range(NCH):
                o = ps[:, nh * NHW:(nh + 1) * NHW]
                for c in range(6):
                    nc.tensor.matmul(
                        o, lhsT=wq_sb[:, c, t * 128:(t + 1) * 128],
                        rhs=qT_sb[:, c, nh * NHW:(nh + 1) * NHW],
                        start=(c == 0), stop=(c == 5))
            nc.vector.tensor_copy(qpT[t][:], ps)
        for t in range(3):
            ps = psA.tile([128, M], f32, tag="proj")
            for mq in range(4):
                o = ps[:, mq * 512:(mq + 1) * 512]
                for c in range(6):
                    nc.tensor.matmul(
                        o, lhsT=wk_sb[:, c, t * 128:(t + 1) * 128],
                        rhs=kvT_sb[:, c, mq * 512:(mq + 1) * 512],
                        start=(c == 0), stop=(c == 5))
            nc.vector.tensor_copy(kT[t][:], ps)
        psA.release()

        psV = tc.alloc_tile_pool(name="psV", bufs=2, space="PSUM")
        for m in range(MC):
            psv = psV.tile([128, DH], f32)
            for c in range(6):
                nc.tensor.matmul(
                    psv[:], lhsT=kvT_sb[:, c, m * 128:(m + 1) * 128],
                    rhs=wv_sb[:, c, :], start=(c == 0), stop=(c == 5))
            nc.vector.tensor_copy(vug[m][:, :, 0:64], psv.rearrange("p (h d) -> p h d", h=HPC))
            nc.vector.memset(vug[m][:, :, 64:65], 1.0)
        psV.release()
        inp.release()
        tmpw.release()

        # ---- phase B: attention ----
        epool = tc.alloc_tile_pool(name="epool", bufs=3)
        mpool = tc.alloc_tile_pool(name="mpool", bufs=3)
        rpool = tc.alloc_tile_pool(name="rpool", bufs=2)
        bpool = tc.alloc_tile_pool(name="bpool", bufs=2)
        xt_pool = tc.alloc_tile_pool(name="xt_pool", bufs=1, space="PSUM")
        st_pool = tc.alloc_tile_pool(name="st_pool", bufs=2, space="PSUM")

        def emit_recip(state, order_after=None):
            # 1/denom = exp(-log(denom)) on ACT (the DVE exact reciprocal is
            # an 8-cycle/element iterative divide, 3.3us per row; Log and Exp
            # share one ACT table set), then a partition-broadcast via SWDGE
            # DMA. Ordered after `order_after` so it never delays the softmax
            # exps at a pair boundary.
            t, nsl, xts = state
            bcss = []
            for j in range(2):
                lg = rpool.tile([1, NHW], f32, tag="lg")
                i1 = nc.scalar.activation(lg[:], xts[j][64:65, :], Log)
                if order_after is not None:
                    tile.add_dep_helper(i1.ins, order_after.ins, sync=False,
                                        reason="norm log after next-pair exp")
                rc = rpool.tile([1, NHW], f32, tag="rc")
                nc.scalar.activation(rc[:], lg[:], Exp, scale=-1.0)
                bcs = bpool.tile([64, NHW], f32, tag="bcs", bufs=4)
                nc.gpsimd.dma_start(
                    out=bcs[:], in_=rc[:, None, :].to_broadcast([1, 64, NHW]))
                bcss.append(bcs)
            return bcss

        def emit_norm_mul(state, bcss, order_after=None):
            # deferred so the DVE multiply never head-of-line blocks the
            # vector queue while its bcs DMA is still in flight
            t, nsl, xts = state
            for j in range(2):
                with nc.allow_low_precision(reason="fp32r rounding is fine"):
                    i = nc.vector.tensor_tensor(
                        xTn[t][j * 64:(j + 1) * 64, nsl],
                        xts[j][0:64, :], bcss[j][:], mult)
                if order_after is not None:
                    tile.add_dep_helper(i.ins, order_after.ins, sync=False,
                                        reason="norm mul after next-pair mask mul")

        pending = None     # (state, bcss_or_None)
        pc = 0             # global pair counter for xt slot rotation
        for ncx in range(2):
            nsl = slice(ncx * NHW, (ncx + 1) * NHW)
            maskt = maskts[ncx]
            for pi in range(3):
                h0 = 2 * pi
                t = pi
                # two accumulators for this head pair; 4 rotating slots so the
                # normalization of pair p overlaps pair p+1's main loop
                xts = [xt_pool.tile([65, NHW], f32, name=f"xt{ncx}_{pi}_{j}",
                                    tag=f"xt{pc % 2}{j}") for j in range(2)]
                pbcss = None
                for m in range(MC):
                    stp = st_pool.tile([128, 2 * NHW], f32, tag="st")
                    for j in range(2):
                        nc.tensor.matmul(
                            stp[:, j * NHW:(j + 1) * NHW],
                            lhsT=kT[t][j * 64:(j + 1) * 64, m * 128:(m + 1) * 128],
                            rhs=qpT[t][j * 64:(j + 1) * 64, nsl],
                            start=True, stop=True)
                    eT = epool.tile([128, 2 * NHW], bf16, tag="eT")
                    for j in range(2):
                        cur_exp = nc.scalar.activation(
                            eT[:, j * NHW:(j + 1) * NHW],
                            stp[:, j * NHW:(j + 1) * NHW], Exp)
                    eTm = mpool.tile([128, 2 * NHW], bf16, tag="eTm")
                    cur_mtt = nc.vector.tensor_tensor(
                        eTm[:], eT[:], maskt[:, m, :, :].rearrange("p j n -> p (j n)"),
                        mult)
                    for j in range(2):
                        nc.tensor.matmul(
                            xts[j][:],
                            lhsT=vug[m][:, h0 + j, :],
                            rhs=eTm[:, j * NHW:(j + 1) * NHW],
                            start=(m == 0), stop=(m == MC - 1))
                    if m == 2 and pending is not None:
                        pending = (pending[0], emit_recip(pending[0], cur_exp))
                    if m == 10 and pending is not None:
                        emit_norm_mul(pending[0], pending[1], cur_mtt)
                        pending = None
                pending = ((t, nsl, xts), None)
                pc += 1
        state, _ = pending
        bcss = emit_recip(state)
        emit_norm_mul(state, bcss)
        st_pool.release()
        xt_pool.release()

        # ---- phase C: output projection (fp32r) ----
        out_sb = persist.tile([128, 8, C], f32)
        psO = tc.alloc_tile_pool(name="psO", bufs=2, space="PSUM")
        out_r = out_d.rearrange("(nt p) c -> p nt c", p=128)
        for nt in range(8):
            pso = psO.tile([128, C], f32)
            for cc, w in ((0, 512), (512, 256)):
                for t in range(3):
                    nc.tensor.matmul(
                        pso[:, cc:cc + w],
                        lhsT=xTn[t][:, nt * 128:(nt + 1) * 128],
                        rhs=wp_sb[:, t, cc:cc + w],
                        start=(t == 0), stop=(t == 2))
            nc.vector.tensor_copy(out_sb[:, nt, :], pso)
            if nt % 2 == 1:
                nc.sync.dma_start(out=out_r[:, nt - 1:nt + 1, :],
                                  in_=out_sb[:, nt - 1:nt + 1, :])
        psO.release()

        for pool in (bpool, rpool, mpool, epool, wp_pool, persist, maskp):
            pool.release()

    if do_split:
        _split_excess_waits(nc)
    return nc


def _get_program():
    if "nc" not in _CACHE:
        _CACHE["nc"] = _build_program()
    return _CACHE["nc"]


def make_in_maps(q, kv, Wq, Wkv, Wproj, bproj, mask):
    import ml_dtypes
    bf = ml_dtypes.bfloat16
    q = np.asarray(q, np.float32)
    kv = np.asarray(kv, np.float32)
    Wq = np.asarray(Wq, np.float32)
    Wkv = np.asarray(Wkv, np.float32)
    Wproj = np.asarray(Wproj, np.float32)
    mask = np.asarray(mask)
    in_maps = []
    for core in range(8):
        b, hg = core // 2, core % 2
        sl = slice(hg * DH, (hg + 1) * DH)
        in_maps.append({
            "qT": np.ascontiguousarray(q[b].T).astype(bf),
            "kvT": np.ascontiguousarray(kv[b].T).astype(bf),
            "wqT": np.ascontiguousarray(Wq[sl, :].T * SCALE).astype(bf),
            "wkT": np.ascontiguousarray(Wkv[sl, :].T).astype(bf),
            "wvT": np.ascontiguousarray(Wkv[C + hg * DH:C + (hg + 1) * DH, :].T).astype(bf),
            "wpT": np.ascontiguousarray(Wproj[:, sl].T).astype(bf),
            "maskT": np.ascontiguousarray(mask[b, 0].T).astype(bf),
        })
    return in_maps


def gather(results, bproj):
    bproj = np.asarray(bproj, np.float32)
    out = np.empty((B, N, C), np.float32)
    for b in range(B):
        out[b] = results[2 * b]["outp"] + results[2 * b + 1]["outp"] + bproj
    return out


def kernel(q, kv, Wq, Wkv, Wproj, bproj, mask):
    from concourse.bass_utils import run_bass_kernel_spmd
    nc = _get_program()
    in_maps = make_in_maps(q, kv, Wq, Wkv, Wproj, bproj, mask)
    res = run_bass_kernel_spmd(nc, in_maps, list(range(8)))
    return gather(res.results, bproj)


# revision 25
# speedup vs baseline: 1.0926x; 1.0926x over previous
"""Trainium2 Bass kernel for nn_Attention_CA (cross-attention, B=4 N=1024 M=2048 C=768 H=12).

Sharding: 8 cores = batch(4) x head-group(2). Each core handles one batch
element and 6 heads; the output projection is row-split per head-group so
each core produces a partial [N, C] output; the host sums the two partials
per batch and adds the bias.

Per-core dataflow (everything transposed so the softmax denominator comes
out of the PV matmul via a ones-column in v and no on-chip transposes are
needed):
  qpT[dd,n]  = q @ WqT (pre-scaled), kT[dd,m], v[m,dd]  (PE, bf16 in / f32 acc)
  sT[m,n]    = kT_h^T qpT_h  [128m x 512n] tiles        (PE bf16, f32 PSUM,
               2 heads packed in PE row-groups 0-63/64-127)
  eT         = exp(sT)                                  (ACT, PSUM->SBUF, bf16)
  eTm        = eT * maskT                               (DVE, bf16 2x mode)
  xT_aug     = [v|1]^T eTm  (row 64 = softmax denom)    (PE bf16, f32 acc)
  xTn        = xT * bcast(1/denom)                      (DVE approx-recip +
                                                         SWDGE partition bcast)
  out_part   = xTn^T @ WprojT_hg                        (PE, fp32r)
"""

import numpy as np

DIM = 768
NUM_HEADS = 12
HEAD_DIM = DIM // NUM_HEADS
SCALE = HEAD_DIM ** -0.5
B, N, M, C = 4, 1024, 2048, DIM
HPC = 6          # heads per core
DH = HPC * HEAD_DIM  # 384
NCH = 2          # n halves
NHW = N // NCH   # 512
MC = M // 128    # 16 m chunks

_CACHE = {}


def _apply_tile_patch():
    """Walrus in this container rejects >1 sem wait per instruction; the
    TileContext tail drain attaches the whole global clock. Spread the waits
    across a chain of sync nops instead."""
    import concourse.tile as _tile
    from concourse.vector_clock import ScopedClock, VectorClock

    if getattr(_tile.TileContext, "_drain_patched", False):
        return

    def _split_drain_and_barrier(self, tick_clock, wait_clock):
        nc = self.nc
        vc = tick_clock.global_clock
        ticks = list(vc)
        n = len(ticks)
        for i, t in enumerate(ticks):
            if t > 0:
                sub = [0] * n
                sub[i] = t
                nop = nc.sync.nop(nofuse=True, hint=f"tail_wait_{i}")
                wait_clock.add_sem_waits(nop.ins, ScopedClock({None: VectorClock(sub)}))
        nc.sync.drain()
        nc.all_engine_barrier()
        assert self.sems is not None
        popped = nc._tile_sem_poison_stack.pop()
        assert popped is self._sem_poison
        nc.clear_and_free_semaphores(list(self.sems.allocated().values()))
        nc.all_engine_barrier()

    _tile.TileContext._drain_and_barrier = _split_drain_and_barrier
    _tile.TileContext._drain_patched = True


def _split_excess_waits(nc):
    """This container's walrus accepts at most 1 sync wait per instruction
    (and none on Matmult); move excess waits onto same-engine NoOps."""
    import concourse.mybir as mybir
    for fn in nc.m.functions:
        for bb in fn.blocks:
            insts = bb.instructions
            out = []
            changed = False
            for ins in insts:
                si = ins.sync_info
                maxw = 0 if isinstance(ins, mybir.InstMatmult) else 1
                if si is not None and len(si.on_wait) > maxw:
                    waits = list(si.on_wait)
                    if maxw == 0:
                        extra, keep = waits, []
                    else:
                        extra, keep = waits[:-maxw], waits[-maxw:]
                    for i, w in enumerate(extra):
                        nop = mybir.InstNoOp(name=f"{ins.name}-w{i}", ins=[], outs=[])
                        nop.engine = ins.engine
                        nop.sync_info = mybir.SyncInfo(on_wait=[w], on_update=[])
                        out.append(nop)
                    ins.sync_info = mybir.SyncInfo(
                        on_wait=keep, on_update=list(si.on_update))
                    changed = True
                out.append(ins)
            if changed:
                bb.instructions[:] = out
    return nc


def _build_program(do_split=True):
    import concourse.bass as bass
    import concourse.mybir as mybir
    import concourse.tile as tile

    _apply_tile_patch()

    f32 = mybir.dt.float32
    f32r = mybir.dt.float32r
    bf16 = mybir.dt.bfloat16
    Exp = mybir.ActivationFunctionType.Exp
    Log = mybir.ActivationFunctionType.Ln
    mult = mybir.AluOpType.mult

    nc = bass.Bass()
    qT_d = nc.declare_dram_parameter("qT", [C, N], bf16, isOutput=False)
    kvT_d = nc.declare_dram_parameter("kvT", [C, M], bf16, isOutput=False)
    wq_d = nc.declare_dram_parameter("wqT", [C, DH], bf16, isOutput=False)
    wk_d = nc.declare_dram_parameter("wkT", [C, DH], bf16, isOutput=False)
    wv_d = nc.declare_dram_parameter("wvT", [C, DH], bf16, isOutput=False)
    wp_d = nc.declare_dram_parameter("wpT", [DH, C], bf16, isOutput=False)
    mask_d = nc.declare_dram_parameter("maskT", [M, N], bf16, isOutput=False)
    out_d = nc.declare_dram_parameter("outp", [N, C], f32, isOutput=True)

    with tile.TileContext(nc) as tc:
        # ---- persistent SBUF ----
        maskp = tc.alloc_tile_pool(name="maskp", bufs=1)
        persist = tc.alloc_tile_pool(name="persist", bufs=1)
        wp_pool = tc.alloc_tile_pool(name="wp_pool", bufs=1)
        tmpw = tc.alloc_tile_pool(name="tmpw", bufs=1)
        inp = tc.alloc_tile_pool(name="inp", bufs=1)

        # preload the Exp/Ln ACT table set off the critical path
        warmt = persist.tile([1, 8], f32)
        nc.vector.memset(warmt[:], 0.0)
        nc.scalar.activation(warmt[:], warmt[:], Exp)

        wq_sb = tmpw.tile([128, 6, DH], bf16)
        wk_sb = tmpw.tile([128, 6, DH], bf16)
        wv_sb = tmpw.tile([128, 6, DH], bf16)
        wp_sb = wp_pool.tile([128, 3, C], bf16)
        qT_sb = inp.tile([128, 6, N], bf16)
        kvT_sb = inp.tile([128, 6, M], bf16)

        nc.sync.dma_start(out=wq_sb[:], in_=wq_d.rearrange("(c p) d -> p c d", p=128))
        nc.sync.dma_start(out=qT_sb[:], in_=qT_d.rearrange("(c p) n -> p c n", p=128))
        nc.sync.dma_start(out=wk_sb[:], in_=wk_d.rearrange("(c p) d -> p c d", p=128))
        nc.sync.dma_start(out=wv_sb[:], in_=wv_d.rearrange("(c p) d -> p c d", p=128))
        nc.sync.dma_start(out=kvT_sb[:], in_=kvT_d.rearrange("(c p) m -> p c m", p=128))
        maskts = []
        for ncx in range(2):
            mt = maskp.tile([128, MC, 2, NHW], bf16, name=f"maskt{ncx}")
            for j in range(2):
                nc.sync.dma_start(
                    out=mt[:, :, j, :],
                    in_=mask_d.rearrange("(mc p) n -> p mc n", p=128)[
                        :, :, ncx * NHW:(ncx + 1) * NHW])
            maskts.append(mt)
        nc.sync.dma_start(out=wp_sb[:], in_=wp_d.rearrange("(t p) c -> p t c", p=128))

        qpT = [persist.tile([128, N], bf16, name=f"qpT{t}") for t in range(3)]
        kT = [persist.tile([128, M], bf16, name=f"kT{t}") for t in range(3)]
        vug = [persist.tile([128, HPC, 65], bf16, name=f"vug{m}") for m in range(MC)]
        xTn = [persist.tile([128, N], bf16, name=f"xTn{t}") for t in range(3)]

        # ---- phase A: projections (bf16 in, f32 accumulate) ----
        psA = tc.alloc_tile_pool(name="psA", bufs=2, space="PSUM")
        for t in range(3):
            ps = psA.tile([128, N], f32, tag="proj", padded_shape=[128, M])
            for nh in range(NCH):
                o = ps[:, nh * NHW:(nh + 1) * NHW]
                for c in range(6):
                    nc.tensor.matmul(
                        o, lhsT=wq_sb[:, c, t * 128:(t + 1) * 128],
                        rhs=qT_sb[:, c, nh * NHW:(nh + 1) * NHW],
                        start=(c == 0), stop=(c == 5))
            nc.vector.tensor_copy(qpT[t][:], ps)
        for t in range(3):
            ps = psA.tile([128, M], f32, tag="proj")
            for mq in range(4):
                o = ps[:, mq * 512:(mq + 1) * 512]
                for c in range(6):
                    nc.tensor.matmul(
                        o, lhsT=wk_sb[:, c, t * 128:(t + 1) * 128],
                        rhs=kvT_sb[:, c, mq * 512:(mq + 1) * 512],
                        start=(c == 0), stop=(c == 5))
            nc.vector.tensor_copy(kT[t][:], ps)
        psA.release()

        psV = tc.alloc_tile_pool(name="psV", bufs=2, space="PSUM")
        for m in range(MC):
            psv = psV.tile([128, DH], f32)
            for c in range(6):
                nc.tensor.matmul(
                    psv[:], lhsT=kvT_sb[:, c, m * 128:(m + 1) * 128],
                    rhs=wv_sb[:, c, :], start=(c == 0), stop=(c == 5))
            nc.vector.tensor_copy(vug[m][:, :, 0:64], psv.rearrange("p (h d) -> p h d", h=HPC))
            nc.vector.memset(vug[m][:, :, 64:65], 1.0)
        psV.release()
        inp.release()
        tmpw.release()

        # ---- phase B: attention ----
        epool = tc.alloc_tile_pool(name="epool", bufs=3)
        mpool = tc.alloc_tile_pool(name="mpool", bufs=3)
        rpool = tc.alloc_tile_pool(name="rpool", bufs=2)
        bpool = tc.alloc_tile_pool(name="bpool", bufs=2)
        xt_pool = tc.alloc_tile_pool(name="xt_pool", bufs=1, space="PSUM")
        st_pool = tc.alloc_tile_pool(name="st_pool", bufs=2, space="PSUM")

        def emit_recip(state, order_after=None):
            # 1/denom = exp(-log(denom)) on ACT (the DVE exact reciprocal is
            # an 8-cycle/element iterative divide, 3.3us per row; Log and Exp
            # share one ACT table set), then a partition-broadcast via SWDGE
            # DMA. Ordered after `order_after` so it never delays the softmax
            # exps at a pair boundary.
            t, nsl, xts = state
            bcss = []
            for j in range(2):
                lg = rpool.tile([1, NHW], f32, tag="lg")
                i1 = nc.scalar.activation(lg[:], xts[j][64:65, :], Log)
                if order_after is not None:
                    tile.add_dep_helper(i1.ins, order_after.ins, sync=False,
                                        reason="norm log after next-pair exp")
                rc = rpool.tile([1, NHW], f32, tag="rc")
                nc.scalar.activation(rc[:], lg[:], Exp, scale=-1.0)
                bcs = bpool.tile([64, NHW], f32, tag="bcs", bufs=4)
                nc.gpsimd.dma_start(
                    out=bcs[:], in_=rc[:, None, :].to_broadcast([1, 64, NHW]))
                bcss.append(bcs)
            return bcss

        def emit_norm_mul(state, bcss, order_after=None):
            # deferred so the DVE multiply never head-of-line blocks the
            # vector queue while its bcs DMA is still in flight
            t, nsl, xts = state
            for j in range(2):
                with nc.allow_low_precision(reason="fp32r rounding is fine"):
                    i = nc.vector.tensor_tensor(
                        xTn[t][j * 64:(j + 1) * 64, nsl],
                        xts[j][0:64, :], bcss[j][:], mult)
                if order_after is not None:
                    tile.add_dep_helper(i.ins, order_after.ins, sync=False,
                                        reason="norm mul after next-pair mask mul")

        pending = None     # (state, bcss_or_None)
        pc = 0             # global pair counter for xt slot rotation
        for ncx in range(2):
            nsl = slice(ncx * NHW, (ncx + 1) * NHW)
            maskt = maskts[ncx]
            for pi in range(3):
                h0 = 2 * pi
                t = pi
                # two accumulators for this head pair; 4 rotating slots so the
                # normalization of pair p overlaps pair p+1's main loop
                xts = [xt_pool.tile([65, NHW], f32, name=f"xt{ncx}_{pi}_{j}",
                                    tag=f"xt{pc % 2}{j}") for j in range(2)]
                pbcss = None
                for m in range(MC):
                    stp = st_pool.tile([128, 2 * NHW], f32, tag="st")
                    for j in range(2):
                        nc.tensor.matmul(
                            stp[:, j * NHW:(j + 1) * NHW],
                            lhsT=kT[t][j * 64:(j + 1) * 64, m * 128:(m + 1) * 128],
                            rhs=qpT[t][j * 64:(j + 1) * 64, nsl],
                            start=True, stop=True)
                    eT = epool.tile([128, 2 * NHW], bf16, tag="eT")
                    for j in range(2):
                        cur_exp = nc.scalar.activation(
                            eT[:, j * NHW:(j + 1) * NHW],
                            stp[:, j * NHW:(j + 1) * NHW], Exp)
                    eTm = mpool.tile([128, 2 * NHW], bf16, tag="eTm")
                    cur_mtt = nc.vector.tensor_tensor(
                        eTm[:], eT[:], maskt[:, m, :, :].rearrange("p j n -> p (j n)"),
                        mult)
                    for j in range(2):
                        nc.tensor.matmul(
                            xts[j][:],
                            lhsT=vug[m][:, h0 + j, :],
                            rhs=eTm[:, j * NHW:(j + 1) * NHW],
                            start=(m == 0), stop=(m == MC - 1))
                    if m == 2 and pending is not None:
                        pending = (pending[0], emit_recip(pending[0], cur_exp))
                    if m == 10 and pending is not None:
                        emit_norm_mul(pending[0], pending[1], cur_mtt)
                        pending = None
                pending = ((t, nsl, xts), None)
                pc += 1
        state, _ = pending
        bcss = emit_recip(state)
        emit_norm_mul(state, bcss)
        st_pool.release()
        xt_pool.release()

        # ---- phase C: output projection (fp32r) ----
        out_sb = persist.tile([128, 8, C], f32)
        psO = tc.alloc_tile_pool(name="psO", bufs=2, space="PSUM")
        out_r = out_d.rearrange("(nt p) c -> p nt c", p=128)
        for nt in range(8):
            pso = psO.tile([128, C], f32)
            for cc, w in ((0, 512), (512, 256)):
                for t in range(3):
                    nc.tensor.matmul(
                        pso[:, cc:cc + w],
                        lhsT=xTn[t][:, nt * 128:(nt + 1) * 128],
                        rhs=wp_sb[:, t, cc:cc + w],
                        start=(t == 0), stop=(t == 2))
            nc.vector.tensor_copy(out_sb[:, nt, :], pso)
            if nt % 2 == 1:
                nc.sync.dma_start(out=out_r[:, nt - 1:nt + 1, :],
                                  in_=out_sb[:, nt - 1:nt + 1, :])
        psO.release()

        for pool in (bpool, rpool, mpool, epool, wp_pool, persist, maskp):
            pool.release()

    if do_split:
        _split_excess_waits(nc)
    return nc


def _get_program():
    if "nc" not in _CACHE:
        _CACHE["nc"] = _build_program()
    return _CACHE["nc"]


def make_in_maps(q, kv, Wq, Wkv, Wproj, bproj, mask):
    import ml_dtypes
    bf = ml_dtypes.bfloat16
    q = np.asarray(q, np.float32)
    kv = np.asarray(kv, np.float32)
    Wq = np.asarray(Wq, np.float32)
    Wkv = np.asarray(Wkv, np.float32)
    Wproj = np.asarray(Wproj, np.float32)
    mask = np.asarray(mask)
    in_maps = []
    for core in range(8):
        b, hg = core // 2, core % 2
        sl = slice(hg * DH, (hg + 1) * DH)
        in_maps.append({
            "qT": np.ascontiguousarray(q[b].T).astype(bf),
            "kvT": np.ascontiguousarray(kv[b].T).astype(bf),
            "wqT": np.ascontiguousarray(Wq[sl, :].T * SCALE).astype(bf),
            "wkT": np.ascontiguousarray(Wkv[sl, :].T).astype(bf),
            "wvT": np.ascontiguousarray(Wkv[C + hg * DH:C + (hg + 1) * DH, :].T).astype(bf),
            "wpT": np.ascontiguousarray(Wproj[:, sl].T).astype(bf),
            "maskT": np.ascontiguousarray(mask[b, 0].T).astype(bf),
        })
    return in_maps


def gather(results, bproj):
    bproj = np.asarray(bproj, np.float32)
    out = np.empty((B, N, C), np.float32)
    for b in range(B):
        out[b] = results[2 * b]["outp"] + results[2 * b + 1]["outp"] + bproj
    return out


def kernel(q, kv, Wq, Wkv, Wproj, bproj, mask):
    from concourse.bass_utils import run_bass_kernel_spmd
    nc = _get_program()
    in_maps = make_in_maps(q, kv, Wq, Wkv, Wproj, bproj, mask)
    res = run_bass_kernel_spmd(nc, in_maps, list(range(8)))
    return gather(res.results, bproj)
